# revision 20
# baseline (speedup 1.0000x reference)
"""Mistral sliding-window attention (B=2, S=2048, H=4096, 32 q-heads / 8 kv-heads,
head_dim=128, window=1024) on 8 Trainium2 NeuronCores.

Sharding: tensor-parallel over heads. Core c owns q-heads [4c, 4c+4) and kv-head c:
  Wq rows [512c, 512c+512), Wk/Wv rows [128c, 128c+128), Wo cols [512c, 512c+512).
Each core computes a full-shape partial output (its heads' contribution through
Wo, in bf16); the host sums the 8 partials in f32 (standard TP unshard).

Per-core kernel (all-bf16 matmuls):
  Phase A: QKV projections from X.T, RoPE fused on the drain (cross-partition
           DVE shifts for rotate_half; D^-0.25 folded into the cos/sin tables
           for both Q and K). Q, K^T, V (PE-transposed) stay resident in SBUF.
           Wq is DMA'd per h-chunk on the vector queue so the first matmul
           starts immediately; X panels own the sync queue.
  Phase B+C per (batch, 256-token q-tile), transposed layout [k, (head, q)]:
           scores for 8 full k-blocks at full width; the two triangular edge
           blocks (diagonal kb=2t+1, window edge kb=2t-8) run at half width
           via strided APs and share one slot of the per-group P^T mega-tile
           (their live halves are complementary). exp on ACT -> bf16 P^T.
           ctx^T accumulates V^T P^T (edges strided, half width). The softmax
           denominator is a strided log-tree DVE sum over the <=9 P^T slots
           plus one ones-matmul (reduces partitions + broadcasts). Fast
           reciprocal + one merged normalize mul, then the output projection
           for those 256 tokens, bf16 partial out.
"""

import math
import sys

sys.path.insert(0, "/opt/trn_rl_repo")

import ml_dtypes
import numpy as np

import concourse.bass as bass
import concourse.mybir as mybir
import concourse.tile as tile
from concourse import bacc
from concourse.bass_utils import run_bass_kernel_spmd

# Problem constants (hardcoded per contract)
B, S, H = 2, 2048, 4096
N_HEADS, N_KV_HEADS, D = 32, 8, 128
WINDOW = 1024
ROPE_THETA = 10000.0
N_CORES = 8
HPC = N_HEADS // N_CORES          # q heads per core = 4
QD = HPC * D                      # per-core q projection dim = 512
T = B * S                         # flattened tokens = 4096

PW = 512                          # phase-A token panel width
QT = 256                          # phase-B query tile width (2 q-blocks)
NEG = -1.0e30

F32 = mybir.dt.float32
BF16 = mybir.dt.bfloat16
AF = mybir.ActivationFunctionType
ALU = mybir.AluOpType

_NC_CACHE = None


def build_nc():
    """Build (once) the single SPMD Bass program all 8 cores run."""
    global _NC_CACHE
    if _NC_CACHE is not None:
        return _NC_CACHE

    nc = bacc.Bacc(None)

    xt_d = nc.dram_tensor("xt", [H, T], BF16, kind="ExternalInput")
    wqt_d = nc.dram_tensor("wqt", [H, QD], BF16, kind="ExternalInput")
    wkvt_d = nc.dram_tensor("wkvt", [H, 2 * D], BF16, kind="ExternalInput")
    wot_d = nc.dram_tensor("wot", [QD, H], BF16, kind="ExternalInput")
    cos_d = nc.dram_tensor("cosb", [D, T], BF16, kind="ExternalInput")
    sin_d = nc.dram_tensor("sinb", [D, T], BF16, kind="ExternalInput")
    mask_d = nc.dram_tensor("masks", [2, D, 2, D], F32, kind="ExternalInput")
    iden_d = nc.dram_tensor("ident", [D, D], BF16, kind="ExternalInput")
    onesb_d = nc.dram_tensor("onesb", [D, D], BF16, kind="ExternalInput")
    out_d = nc.dram_tensor("out", [T, H], BF16, kind="ExternalOutput")

    HC = H // 128                 # 32 h-chunks
    NPAN = T // PW                # 8 token panels
    NQT = S // QT                 # 8 q-tiles per batch
    QC = QD // 128                # 4 qd chunks == heads per core
    NSL = 9                       # P^T mega-tile slots: 1 edge + 8 full

    with tile.TileContext(nc) as tc, nc.allow_low_precision(reason="mixed dtypes"):
        with (
            tc.tile_pool(name="persist", bufs=1) as ppool,
            tc.tile_pool(name="wopool", bufs=1) as wopool,
            tc.tile_pool(name="bpool", bufs=1) as bpool,
        ):
            # K^T (rope'd), V natural-layout and Q (all bf16) stay in SBUF
            kt_full = ppool.tile([D, T], BF16)
            vnat = ppool.tile([128, T // 128, D], BF16)
            qsb = ppool.tile([D, B, HPC, S], BF16)
            # phase-B constants prefetched up front (overlap with phase A)
            mask_s = bpool.tile([D, 2, 2, D], F32)
            nc.gpsimd.dma_start(mask_s[:], mask_d[:].rearrange("m p t q -> p m t q"))
            onesb_s = bpool.tile([D, D], BF16)
            nc.gpsimd.dma_start(onesb_s[:], onesb_d[:])

            # ---------------- Phase A: QKV projections + RoPE ----------------
            with (
                tc.tile_pool(name="wpool", bufs=1) as wpool,
                tc.tile_pool(name="xpool", bufs=12) as xpool,
                tc.tile_pool(name="cspool", bufs=1) as cspool,
                tc.tile_pool(name="apool", bufs=2) as apool,
                tc.tile_pool(name="psA", bufs=1, space="PSUM") as psA,
            ):
                # weights DMA'd per h-chunk, interleaved with panel-0 X chunks
                # (sync queue) and on gpsimd (wk|wv) so each chunk lands just
                # ahead of its matmuls; first matmul starts almost immediately
                wq_s = wpool.tile([128, HC, QD], BF16)
                wkv_s = wpool.tile([128, HC, 2 * D], BF16)
                cos_s = cspool.tile([D, T], BF16)
                sin_s = cspool.tile([D, T], BF16)
                iden_s = cspool.tile([D, D], BF16)
                nc.gpsimd.dma_start(iden_s[:], iden_d[:])
                wo_s = wopool.tile([128, QC, H], BF16)

                for p in range(NPAN):
                    tok = slice(p * PW, (p + 1) * PW)
                    bp = (p * PW) // S        # batch this panel belongs to
                    ps_q = [
                        psA.tile([128, PW], F32, tag=f"psq{j}", name=f"psq{j}")
                        for j in range(HPC)
                    ]
                    ps_k = psA.tile([128, PW], F32, tag="psk")
                    ps_v = psA.tile([128, PW], F32, tag="psv")
                    for hc in range(HC):
                        if p == 0:
                            nc.scalar.dma_start(
                                wq_s[:, hc, :], wqt_d[hc * 128 : (hc + 1) * 128, :]
                            )
                            nc.gpsimd.dma_start(
                                wkv_s[:, hc, :], wkvt_d[hc * 128 : (hc + 1) * 128, :]
                            )
                        x_c = xpool.tile([128, PW], BF16, tag="x_c")
                        nc.sync.dma_start(
                            x_c[:], xt_d[hc * 128 : (hc + 1) * 128, tok]
                        )
                        st, sp = hc == 0, hc == HC - 1
                        for j in range(HPC):
                            nc.tensor.matmul(
                                ps_q[j][:],
                                wq_s[:, hc, j * 128 : (j + 1) * 128],
                                x_c[:],
                                start=st,
                                stop=sp,
                            )
                        nc.tensor.matmul(ps_k[:], wkv_s[:, hc, 0:D], x_c[:], start=st, stop=sp)
                        nc.tensor.matmul(ps_v[:], wkv_s[:, hc, D:], x_c[:], start=st, stop=sp)
                    if p == 0:
                        # scalar queue after the wq chunks: rope tables (needed
                        # ~40us in) then the Wo prefetch (needed ~350us in)
                        nc.scalar.dma_start(cos_s[:], cos_d[:])
                        nc.scalar.dma_start(sin_s[:], sin_d[:])
                        nc.scalar.dma_start(
                            wo_s[:], wot_d[:].rearrange("(qc p) hh -> p qc hh", p=128)
                        )

                    # RoPE in bf16: psum drain copy first (frees the accumulator),
                    # then 2x/4x-mode bf16 DVE ops. rot(x)[p<64] = -x[p+64]; [p>=64] = x[p-64]
                    # For the LAST panel the math chains run on GpSimd so the
                    # DVE queue is empty when phase B's mask adds arrive.
                    last = p == NPAN - 1
                    eng = nc.gpsimd if last else nc.vector

                    def rope_math(ps_t, out_ap, sb=None):
                        if sb is None:
                            sb = apool.tile([128, PW], BF16, tag="ropesb", bufs=6, name="ropesb")
                            nc.vector.tensor_copy(sb[:], ps_t[:])
                        rot = apool.tile([128, PW], BF16, tag="rot", bufs=3, name="rot")
                        eng.tensor_scalar_mul(rot[0:64, :], sb[64:128, :], -1.0)
                        eng.tensor_copy(rot[64:128, :], sb[0:64, :])
                        prod = apool.tile([128, PW], BF16, tag="prod", bufs=3, name="prod")
                        eng.tensor_mul(out=prod[:], in0=sb[:], in1=cos_s[:, tok])
                        eng.tensor_mul(out=rot[:], in0=rot[:], in1=sin_s[:, tok])
                        eng.tensor_add(out=out_ap, in0=prod[:], in1=rot[:])

                    lo = p * PW - bp * S
                    if last:
                        # drain all five accumulators on DVE first, then the
                        # gpsimd math chains
                        sbs = []
                        for ps_t in ps_q + [ps_k]:
                            sb = apool.tile([128, PW], BF16, tag="ropesb", bufs=6, name="ropesb")
                            nc.vector.tensor_copy(sb[:], ps_t[:])
                            sbs.append(sb)
                        rope_math(None, kt_full[:, tok], sb=sbs[HPC])
                        for j in range(HPC):
                            rope_math(None, qsb[:, bp, j, lo : lo + PW], sb=sbs[j])
                    else:
                        for j in range(HPC):
                            rope_math(ps_q[j], qsb[:, bp, j, lo : lo + PW])
                        rope_math(ps_k, kt_full[:, tok])

                    # V natural layout via PE transpose (ACT does the psum drain)
                    v_sb = apool.tile([128, PW], BF16, tag="v_sb")
                    nc.scalar.copy(v_sb[:], ps_v[:])
                    for blk in range(PW // 128):
                        tp = psA.tile([D, D], BF16, tag="tp", bufs=2, name="tp")
                        nc.tensor.transpose(
                            tp[:], v_sb[:, blk * 128 : (blk + 1) * 128], iden_s[:]
                        )
                        nc.vector.tensor_copy(vnat[:, p * (PW // 128) + blk, :], tp[:])

            # ------------- Phase B+C: attention + output projection -------------
            # Software-pipelined tiles: tile t's output projection is emitted
            # between tile t+1's scores and ctx so the PE never waits on the
            # exp (ACT) / den-tree+reciprocal (DVE) chains.
            with (
                tc.tile_pool(name="epool", bufs=4) as epool,
                tc.tile_pool(name="spool", bufs=2) as spool,
                tc.tile_pool(name="npool", bufs=2) as npool,
                tc.tile_pool(name="cxpool", bufs=6) as cxpool,
                tc.tile_pool(name="opool", bufs=3) as opool,
                tc.tile_pool(name="psB", bufs=1, space="PSUM") as psB,
            ):
                LH = slice(0, 128)            # q-block 2t cols within a head
                RH = slice(128, QT)           # q-block 2t+1 cols within a head

                def emit_outproj(pend):
                    pb, pt, pctx = pend
                    for tl in range(QT // 128):
                        tok0 = pb * S + pt * QT + tl * 128
                        o_all = opool.tile([128, H], BF16, tag="o_all", bufs=3)
                        for hb in range(H // 512):
                            ps_o = psB.tile([128, 512], F32, tag="ps_o", bufs=2, name="ps_o")
                            for qc in range(QC):
                                nc.tensor.matmul(
                                    ps_o[:],
                                    pctx[qc // 2][:, qc % 2, tl * 128 : (tl + 1) * 128],
                                    wo_s[:, qc, hb * 512 : (hb + 1) * 512],
                                    start=(qc == 0),
                                    stop=(qc == QC - 1),
                                )
                            osl = o_all[:, hb * 512 : (hb + 1) * 512]
                            # drain split ~11 DVE / 5 ACT per tile
                            if (tl * 8 + hb) % 3 == 2:
                                nc.scalar.copy(osl, ps_o[:])
                            else:
                                nc.vector.tensor_copy(osl, ps_o[:])
                        nc.gpsimd.dma_start(out_d[tok0 : tok0 + 128, :], o_all[:])

                pending = None
                for b in range(B):
                    for t in range(NQT):
                        qsl = slice(t * QT, (t + 1) * QT)
                        kb_diag = 2 * t + 1
                        kb_far = 2 * t - 8          # may be < 0 (absent)
                        full_kbs = list(range(max(0, kb_far + 1), kb_diag))
                        nsl = 1 + len(full_kbs)
                        vbase = (b * S) // 128
                        qps = [qsb[:, b, 2 * g : 2 * g + 2, qsl] for g in range(HPC // 2)]
                        e_alls = []

                        # -- scores + masks + exp for both head pairs --
                        for g in range(HPC // 2):
                            qp = qps[g]
                            e_all = epool.tile([D, NSL, 2, QT], BF16, tag="e_all", name="e_all")
                            e_alls.append(e_all)
                            for i, kb in enumerate(full_kbs):
                                s_ps = psB.tile([D, 2, QT], F32, tag="sc", bufs=4, name="s_ps")
                                nc.tensor.matmul(
                                    s_ps[:],
                                    kt_full[:, b * S + kb * 128 : b * S + (kb + 1) * 128],
                                    qp,
                                    start=True,
                                    stop=True,
                                )
                                if kb == 2 * t:
                                    nc.vector.tensor_add(
                                        out=s_ps[:, :, LH], in0=s_ps[:, :, LH],
                                        in1=mask_s[:, 0, :, :],
                                    )
                                elif kb == 2 * t - 7:
                                    nc.vector.tensor_add(
                                        out=s_ps[:, :, RH], in0=s_ps[:, :, RH],
                                        in1=mask_s[:, 1, :, :],
                                    )
                                nc.scalar.activation(e_all[:, 1 + i, :, :], s_ps[:], AF.Exp)

                            # edge k-blocks at half width into shared slot 0:
                            # diag (kb=2t+1) live on RH, far (kb=2t-8) live on LH
                            s_pe = psB.tile([D, 2, QT], F32, tag="sc", bufs=4, name="s_pe")
                            nc.tensor.matmul(
                                s_pe[:, :, RH],
                                kt_full[:, b * S + kb_diag * 128 : b * S + (kb_diag + 1) * 128],
                                qp[:, :, RH],
                                start=True,
                                stop=True,
                            )
                            nc.vector.tensor_add(
                                out=s_pe[:, :, RH], in0=s_pe[:, :, RH],
                                in1=mask_s[:, 0, :, :],
                            )
                            nc.scalar.activation(e_all[:, 0, :, RH], s_pe[:, :, RH], AF.Exp)
                            if kb_far >= 0:
                                s_pf = psB.tile([D, 2, QT], F32, tag="sc", bufs=4, name="s_pf")
                                nc.tensor.matmul(
                                    s_pf[:, :, LH],
                                    kt_full[:, b * S + kb_far * 128 : b * S + (kb_far + 1) * 128],
                                    qp[:, :, LH],
                                    start=True,
                                    stop=True,
                                )
                                nc.vector.tensor_add(
                                    out=s_pf[:, :, LH], in0=s_pf[:, :, LH],
                                    in1=mask_s[:, 1, :, :],
                                )
                                nc.scalar.activation(e_all[:, 0, :, LH], s_pf[:, :, LH], AF.Exp)
                            else:
                                nc.vector.memset(e_all[:, 0, :, LH], 0.0)

                        # -- previous tile's output projection fills the gap --
                        if pending is not None:
                            emit_outproj(pending)

                        # -- ctx accumulation for both head pairs --
                        ctx2s = []
                        for g in range(HPC // 2):
                            e_all = e_alls[g]
                            ctx2 = psB.tile([D, 2, QT], F32, tag="ctx", bufs=2, name="ctx2")
                            ctx2s.append(ctx2)
                            for i, kb in enumerate(full_kbs):
                                nc.tensor.matmul(
                                    ctx2[:], vnat[:, vbase + kb, :], e_all[:, 1 + i, :, :],
                                    start=(i == 0), stop=False,
                                )
                            if kb_far >= 0:
                                nc.tensor.matmul(
                                    ctx2[:, :, LH], vnat[:, vbase + kb_far, :],
                                    e_all[:, 0, :, LH], start=False, stop=False,
                                )
                            nc.tensor.matmul(
                                ctx2[:, :, RH], vnat[:, vbase + kb_diag, :],
                                e_all[:, 0, :, RH], start=False, stop=True,
                            )

                        # -- denominators + normalize for both head pairs --
                        ctx_sbs = [None] * (HPC // 2)
                        for g in range(HPC // 2):
                            e_all = e_alls[g]
                            den2_t = psB.tile([D, 2, QT], F32, tag="sc", bufs=4, name="den2_t")
                            esum = spool.tile([D, (NSL + 1) // 2, 2, QT], BF16, tag="esum", name="esum")
                            cur, n = e_all, nsl
                            while n > 1:
                                h = n // 2
                                nc.vector.tensor_add(
                                    out=esum[:, 0:h], in0=cur[:, 0:h], in1=cur[:, h : 2 * h]
                                )
                                if n % 2:
                                    nc.vector.tensor_add(
                                        out=esum[:, 0:1], in0=esum[:, 0:1],
                                        in1=cur[:, 2 * h : 2 * h + 1],
                                    )
                                cur, n = esum, h
                            nc.tensor.matmul(
                                den2_t[:], onesb_s[:], esum[:, 0, :, :], start=True, stop=True
                            )
                            recf = npool.tile([D, 2, QT], F32, tag="recf", name="recf")
                            nc.vector.reciprocal_approx_fast(recf[:], den2_t[:])
                            ctx_sb = cxpool.tile([D, 2, QT], BF16, tag="ctx_sb", name="ctx_sb")
                            nc.vector.tensor_mul(out=ctx_sb[:], in0=ctx2s[g][:], in1=recf[:])
                            ctx_sbs[g] = ctx_sb

                        pending = (b, t, ctx_sbs)
                emit_outproj(pending)

    nc.finalize()
    _NC_CACHE = nc
    return nc


def _rope_cache_np(position_ids):
    """cos/sin [D, T] transposed rope cache from actual position ids,
    scaled by D^-0.25 (half of 1/sqrt(D) on each of Q and K)."""
    lam = float(D) ** -0.25
    inv_freq = 1.0 / (ROPE_THETA ** (np.arange(0, D, 2, dtype=np.float64) / D))
    cos_parts, sin_parts = [], []
    for b in range(B):
        t = np.asarray(position_ids[b], dtype=np.float64)
        freqs = np.outer(t, inv_freq)                    # [S, D/2]
        emb = np.concatenate([freqs, freqs], axis=-1)    # [S, D]
        cos_parts.append(np.cos(emb).T * lam)
        sin_parts.append(np.sin(emb).T * lam)
    cos = np.ascontiguousarray(np.concatenate(cos_parts, axis=1)).astype(ml_dtypes.bfloat16)
    sin = np.ascontiguousarray(np.concatenate(sin_parts, axis=1)).astype(ml_dtypes.bfloat16)
    return cos, sin


def _mask_tiles_np():
    """[2, 128, 2, 128] additive bias tiles in [k, (head, q)] layout; the same
    mask is duplicated on the head axis so one strided DVE add covers both.

    diag[kl, ql] = 0 if kl <= ql else NEG        (k-block == q-block)
    far[kl, ql]  = 0 if ql <  kl else NEG        (k-block == q-block - 8)
    """
    kl = np.arange(128)[:, None]
    ql = np.arange(128)[None, :]
    diag = np.where(kl <= ql, 0.0, NEG).astype(np.float32)
    far = np.where(ql < kl, 0.0, NEG).astype(np.float32)
    both = np.stack([diag, far])                          # [2, 128, 128]
    return np.repeat(both[:, :, None, :], 2, axis=2).astype(np.float32)


def host_in_maps(hidden_states, Wq, Wk, Wv, Wo, position_ids):
    """Shard + pre-layout the full inputs into 8 per-core input maps."""
    hidden_states = np.asarray(hidden_states, dtype=np.float32)
    Wq = np.asarray(Wq, dtype=np.float32)
    Wk = np.asarray(Wk, dtype=np.float32)
    Wv = np.asarray(Wv, dtype=np.float32)
    Wo = np.asarray(Wo, dtype=np.float32)

    bf = ml_dtypes.bfloat16
    xt = np.ascontiguousarray(hidden_states.reshape(T, H).T).astype(bf)
    cos, sin = _rope_cache_np(np.asarray(position_ids))
    masks = _mask_tiles_np()
    ident = np.eye(D).astype(bf)
    onesb = np.ones((D, D), dtype=bf)

    in_maps = []
    for c in range(N_CORES):
        wqt = np.ascontiguousarray(Wq[c * QD : (c + 1) * QD, :].T).astype(bf)
        wkvt = np.ascontiguousarray(
            np.concatenate(
                [Wk[c * D : (c + 1) * D, :].T, Wv[c * D : (c + 1) * D, :].T], axis=1
            )
        ).astype(bf)
        wot = np.ascontiguousarray(Wo[:, c * QD : (c + 1) * QD].T).astype(bf)
        in_maps.append(
            {
                "xt": xt,
                "wqt": wqt,
                "wkvt": wkvt,
                "wot": wot,
                "cosb": cos,
                "sinb": sin,
                "masks": masks,
                "ident": ident,
                "onesb": onesb,
            }
        )
    return in_maps


def kernel(hidden_states, Wq, Wk, Wv, Wo, position_ids):
    nc = build_nc()
    in_maps = host_in_maps(hidden_states, Wq, Wk, Wv, Wo, position_ids)
    res = run_bass_kernel_spmd(nc, in_maps, core_ids=list(range(N_CORES)))
    total = res.results[0]["out"].astype(np.float32)
    for c in range(1, N_CORES):
        total = total + res.results[c]["out"].astype(np.float32)
    return np.ascontiguousarray(total.reshape(B, S, H), dtype=np.float32)


# revision 21
# speedup vs baseline: 1.0541x; 1.0541x over previous
"""Mistral sliding-window attention (B=2, S=2048, H=4096, 32 q-heads / 8 kv-heads,
head_dim=128, window=1024) on 8 Trainium2 NeuronCores.

Sharding: tensor-parallel over heads. Core c owns q-heads [4c, 4c+4) and kv-head c:
  Wq rows [512c, 512c+512), Wk/Wv rows [128c, 128c+128), Wo cols [512c, 512c+512).
Each core computes a full-shape partial output (its heads' contribution through
Wo, in bf16); the host sums the 8 partials in f32 (standard TP unshard).

Per-core kernel (all-bf16 matmuls):
  Phase A: QKV projections from X.T, RoPE fused on the drain (cross-partition
           DVE shifts for rotate_half; D^-0.25 folded into the cos/sin tables
           for both Q and K). Q, K^T, V (PE-transposed) stay resident in SBUF.
           Wq is DMA'd per h-chunk on the vector queue so the first matmul
           starts immediately; X panels own the sync queue.
  Phase B+C per (batch, 256-token q-tile), transposed layout [k, (head, q)]:
           scores for 8 full k-blocks at full width; the two triangular edge
           blocks (diagonal kb=2t+1, window edge kb=2t-8) run at half width
           via strided APs and share one slot of the per-group P^T mega-tile
           (their live halves are complementary). exp on ACT -> bf16 P^T.
           ctx^T accumulates V^T P^T (edges strided, half width). The softmax
           denominator is a strided log-tree DVE sum over the <=9 P^T slots
           plus one ones-matmul (reduces partitions + broadcasts). Fast
           reciprocal + one merged normalize mul, then the output projection
           for those 256 tokens, bf16 partial out.
"""

import math
import sys

sys.path.insert(0, "/opt/trn_rl_repo")

import ml_dtypes
import numpy as np

import concourse.bass as bass
import concourse.mybir as mybir
import concourse.tile as tile
from concourse import bacc
from concourse.bass_utils import run_bass_kernel_spmd

# Problem constants (hardcoded per contract)
B, S, H = 2, 2048, 4096
N_HEADS, N_KV_HEADS, D = 32, 8, 128
WINDOW = 1024
ROPE_THETA = 10000.0
N_CORES = 8
HPC = N_HEADS // N_CORES          # q heads per core = 4
QD = HPC * D                      # per-core q projection dim = 512
T = B * S                         # flattened tokens = 4096

PW = 512                          # phase-A token panel width
QT = 256                          # phase-B query tile width (2 q-blocks)
NEG = -1.0e30

F32 = mybir.dt.float32
BF16 = mybir.dt.bfloat16
AF = mybir.ActivationFunctionType
ALU = mybir.AluOpType

_NC_CACHE = None


def build_nc():
    """Build (once) the single SPMD Bass program all 8 cores run."""
    global _NC_CACHE
    if _NC_CACHE is not None:
        return _NC_CACHE

    nc = bacc.Bacc(None)

    xt_d = nc.dram_tensor("xt", [H, T], BF16, kind="ExternalInput")
    wqt_d = nc.dram_tensor("wqt", [H, QD], BF16, kind="ExternalInput")
    wkvt_d = nc.dram_tensor("wkvt", [H, 2 * D], BF16, kind="ExternalInput")
    wot_d = nc.dram_tensor("wot", [QD, H], BF16, kind="ExternalInput")
    cos_d = nc.dram_tensor("cosb", [D, T], BF16, kind="ExternalInput")
    sin_d = nc.dram_tensor("sinb", [D, T], BF16, kind="ExternalInput")
    mask_d = nc.dram_tensor("masks", [2, D, 2, D], F32, kind="ExternalInput")
    iden_d = nc.dram_tensor("ident", [D, D], BF16, kind="ExternalInput")
    onesb_d = nc.dram_tensor("onesb", [D, D], BF16, kind="ExternalInput")
    out_d = nc.dram_tensor("out", [T, H], BF16, kind="ExternalOutput")

    HC = H // 128                 # 32 h-chunks
    NPAN = T // PW                # 8 token panels
    NQT = S // QT                 # 8 q-tiles per batch
    QC = QD // 128                # 4 qd chunks == heads per core
    NSL = 9                       # P^T mega-tile slots: 1 edge + 8 full

    with tile.TileContext(nc) as tc, nc.allow_low_precision(reason="mixed dtypes"):
        with (
            tc.tile_pool(name="persist", bufs=1) as ppool,
            tc.tile_pool(name="wopool", bufs=1) as wopool,
            tc.tile_pool(name="bpool", bufs=1) as bpool,
        ):
            # K^T (rope'd), V natural-layout and Q (all bf16) stay in SBUF
            kt_full = ppool.tile([D, T], BF16)
            vnat = ppool.tile([128, T // 128, D], BF16)
            qsb = ppool.tile([D, B, HPC, S], BF16)
            # phase-B constants prefetched up front (overlap with phase A)
            mask_s = bpool.tile([D, 2, 2, D], F32)
            nc.gpsimd.dma_start(mask_s[:], mask_d[:].rearrange("m p t q -> p m t q"))
            onesb_s = bpool.tile([D, D], BF16)
            nc.gpsimd.dma_start(onesb_s[:], onesb_d[:])

            # ---------------- Phase A: QKV projections + RoPE ----------------
            with (
                tc.tile_pool(name="wpool", bufs=1) as wpool,
                tc.tile_pool(name="xpool", bufs=8) as xpool,
                tc.tile_pool(name="cspool", bufs=1) as cspool,
                tc.tile_pool(name="apool", bufs=2) as apool,
                tc.tile_pool(name="psA", bufs=1, space="PSUM") as psA,
            ):
                # weights DMA'd per h-chunk, interleaved with panel-0 X chunks
                # (sync queue) and on gpsimd (wk|wv) so each chunk lands just
                # ahead of its matmuls; first matmul starts almost immediately
                wq_s = wpool.tile([128, HC, QD], BF16)
                wkv_s = wpool.tile([128, HC, 2 * D], BF16)
                cos_s = cspool.tile([D, T], BF16)
                sin_s = cspool.tile([D, T], BF16)
                iden_s = cspool.tile([D, D], BF16)
                nc.gpsimd.dma_start(iden_s[:], iden_d[:])
                wo_s = wopool.tile([128, QC, H], BF16)

                for p in range(NPAN):
                    tok = slice(p * PW, (p + 1) * PW)
                    bp = (p * PW) // S        # batch this panel belongs to
                    ps_q = [
                        psA.tile([128, PW], F32, tag=f"psq{j}", name=f"psq{j}")
                        for j in range(HPC)
                    ]
                    ps_k = psA.tile([128, PW], F32, tag="psk")
                    ps_v = psA.tile([128, PW], F32, tag="psv")
                    for hc in range(HC):
                        if p == 0:
                            nc.scalar.dma_start(
                                wq_s[:, hc, :], wqt_d[hc * 128 : (hc + 1) * 128, :]
                            )
                            nc.gpsimd.dma_start(
                                wkv_s[:, hc, :], wkvt_d[hc * 128 : (hc + 1) * 128, :]
                            )
                        x_c = xpool.tile([128, PW], BF16, tag="x_c")
                        nc.sync.dma_start(
                            x_c[:], xt_d[hc * 128 : (hc + 1) * 128, tok]
                        )
                        st, sp = hc == 0, hc == HC - 1
                        for j in range(HPC):
                            nc.tensor.matmul(
                                ps_q[j][:],
                                wq_s[:, hc, j * 128 : (j + 1) * 128],
                                x_c[:],
                                start=st,
                                stop=sp,
                            )
                        nc.tensor.matmul(ps_k[:], wkv_s[:, hc, 0:D], x_c[:], start=st, stop=sp)
                        nc.tensor.matmul(ps_v[:], wkv_s[:, hc, D:], x_c[:], start=st, stop=sp)
                    if p == 0:
                        # scalar queue after the wq chunks: rope tables (needed
                        # ~40us in) then the Wo prefetch (needed ~350us in)
                        nc.scalar.dma_start(cos_s[:], cos_d[:])
                        nc.scalar.dma_start(sin_s[:], sin_d[:])
                        nc.scalar.dma_start(
                            wo_s[:], wot_d[:].rearrange("(qc p) hh -> p qc hh", p=128)
                        )

                    # RoPE in bf16: psum drain copy first (frees the accumulator),
                    # then 2x/4x-mode bf16 DVE ops. rot(x)[p<64] = -x[p+64]; [p>=64] = x[p-64]
                    def rope_math(ps_t, out_ap):
                        sb = apool.tile([128, PW], BF16, tag="ropesb", bufs=3, name="ropesb")
                        nc.vector.tensor_copy(sb[:], ps_t[:])
                        rot = apool.tile([128, PW], BF16, tag="rot", bufs=3, name="rot")
                        nc.vector.tensor_scalar_mul(rot[0:64, :], sb[64:128, :], -1.0)
                        nc.vector.tensor_copy(rot[64:128, :], sb[0:64, :])
                        prod = apool.tile([128, PW], BF16, tag="prod", bufs=3, name="prod")
                        nc.vector.tensor_mul(out=prod[:], in0=sb[:], in1=cos_s[:, tok])
                        nc.vector.tensor_mul(out=rot[:], in0=rot[:], in1=sin_s[:, tok])
                        nc.vector.tensor_add(out=out_ap, in0=prod[:], in1=rot[:])

                    lo = p * PW - bp * S
                    if p == NPAN - 1:
                        # last panel: K first and Q in reverse so the PSUM
                        # banks phase B's first matmuls land on free earliest
                        rope_math(ps_k, kt_full[:, tok])
                        for j in reversed(range(HPC)):
                            rope_math(ps_q[j], qsb[:, bp, j, lo : lo + PW])
                    else:
                        for j in range(HPC):
                            rope_math(ps_q[j], qsb[:, bp, j, lo : lo + PW])
                        rope_math(ps_k, kt_full[:, tok])

                    # V natural layout via PE transpose (ACT does the psum drain)
                    v_sb = apool.tile([128, PW], BF16, tag="v_sb")
                    nc.scalar.copy(v_sb[:], ps_v[:])
                    for blk in range(PW // 128):
                        tp = psA.tile([D, D], BF16, tag="tp", bufs=2, name="tp")
                        nc.tensor.transpose(
                            tp[:], v_sb[:, blk * 128 : (blk + 1) * 128], iden_s[:]
                        )
                        nc.vector.tensor_copy(vnat[:, p * (PW // 128) + blk, :], tp[:])

            # ------------- Phase B+C: attention + output projection -------------
            # Software-pipelined tiles: tile t's output projection is emitted
            # between tile t+1's scores and ctx so the PE never waits on the
            # exp (ACT) / den-tree+reciprocal (DVE) chains.
            with (
                tc.tile_pool(name="epool", bufs=4) as epool,
                tc.tile_pool(name="spool", bufs=2) as spool,
                tc.tile_pool(name="npool", bufs=2) as npool,
                tc.tile_pool(name="cxpool", bufs=6) as cxpool,
                tc.tile_pool(name="opool", bufs=3) as opool,
                tc.tile_pool(name="psB", bufs=1, space="PSUM") as psB,
            ):
                LH = slice(0, 128)            # q-block 2t cols within a head
                RH = slice(128, QT)           # q-block 2t+1 cols within a head

                def emit_outproj(pend):
                    pb, pt, pctx = pend
                    for tl in range(QT // 128):
                        tok0 = pb * S + pt * QT + tl * 128
                        o_all = opool.tile([128, H], BF16, tag="o_all", bufs=3)
                        for hb in range(H // 512):
                            ps_o = psB.tile([128, 512], F32, tag="ps_o", bufs=2, name="ps_o")
                            for qc in range(QC):
                                nc.tensor.matmul(
                                    ps_o[:],
                                    pctx[qc // 2][:, qc % 2, tl * 128 : (tl + 1) * 128],
                                    wo_s[:, qc, hb * 512 : (hb + 1) * 512],
                                    start=(qc == 0),
                                    stop=(qc == QC - 1),
                                )
                            osl = o_all[:, hb * 512 : (hb + 1) * 512]
                            # drain split ~11 DVE / 5 ACT per tile
                            if (tl * 8 + hb) % 3 == 2:
                                nc.scalar.copy(osl, ps_o[:])
                            else:
                                nc.vector.tensor_copy(osl, ps_o[:])
                        nc.gpsimd.dma_start(out_d[tok0 : tok0 + 128, :], o_all[:])

                pending = None
                for b in range(B):
                    for t in range(NQT):
                        qsl = slice(t * QT, (t + 1) * QT)
                        kb_diag = 2 * t + 1
                        kb_far = 2 * t - 8          # may be < 0 (absent)
                        full_kbs = list(range(max(0, kb_far + 1), kb_diag))
                        nsl = 1 + len(full_kbs)
                        vbase = (b * S) // 128
                        qps = [qsb[:, b, 2 * g : 2 * g + 2, qsl] for g in range(HPC // 2)]
                        e_alls = []

                        # -- scores + masks + exp for both head pairs --
                        for g in range(HPC // 2):
                            qp = qps[g]
                            e_all = epool.tile([D, NSL, 2, QT], BF16, tag="e_all", name="e_all")
                            e_alls.append(e_all)
                            for i, kb in enumerate(full_kbs):
                                s_ps = psB.tile([D, 2, QT], F32, tag="sc", bufs=3, name="s_ps")
                                nc.tensor.matmul(
                                    s_ps[:],
                                    kt_full[:, b * S + kb * 128 : b * S + (kb + 1) * 128],
                                    qp,
                                    start=True,
                                    stop=True,
                                )
                                if kb == 2 * t:
                                    nc.vector.tensor_add(
                                        out=s_ps[:, :, LH], in0=s_ps[:, :, LH],
                                        in1=mask_s[:, 0, :, :],
                                    )
                                elif kb == 2 * t - 7:
                                    nc.vector.tensor_add(
                                        out=s_ps[:, :, RH], in0=s_ps[:, :, RH],
                                        in1=mask_s[:, 1, :, :],
                                    )
                                nc.scalar.activation(e_all[:, 1 + i, :, :], s_ps[:], AF.Exp)

                            # edge k-blocks at half width into shared slot 0:
                            # diag (kb=2t+1) live on RH, far (kb=2t-8) live on LH
                            s_pe = psB.tile([D, 2, QT], F32, tag="sc", bufs=3, name="s_pe")
                            nc.tensor.matmul(
                                s_pe[:, :, RH],
                                kt_full[:, b * S + kb_diag * 128 : b * S + (kb_diag + 1) * 128],
                                qp[:, :, RH],
                                start=True,
                                stop=True,
                            )
                            nc.vector.tensor_add(
                                out=s_pe[:, :, RH], in0=s_pe[:, :, RH],
                                in1=mask_s[:, 0, :, :],
                            )
                            nc.scalar.activation(e_all[:, 0, :, RH], s_pe[:, :, RH], AF.Exp)
                            if kb_far >= 0:
                                s_pf = psB.tile([D, 2, QT], F32, tag="sc", bufs=3, name="s_pf")
                                nc.tensor.matmul(
                                    s_pf[:, :, LH],
                                    kt_full[:, b * S + kb_far * 128 : b * S + (kb_far + 1) * 128],
                                    qp[:, :, LH],
                                    start=True,
                                    stop=True,
                                )
                                nc.vector.tensor_add(
                                    out=s_pf[:, :, LH], in0=s_pf[:, :, LH],
                                    in1=mask_s[:, 1, :, :],
                                )
                                nc.scalar.activation(e_all[:, 0, :, LH], s_pf[:, :, LH], AF.Exp)
                            else:
                                nc.vector.memset(e_all[:, 0, :, LH], 0.0)

                        # -- previous tile's output projection fills the gap --
                        if pending is not None:
                            emit_outproj(pending)

                        # -- ctx accumulation for both head pairs --
                        ctx2s = []
                        for g in range(HPC // 2):
                            e_all = e_alls[g]
                            ctx2 = psB.tile([D, 2, QT], F32, tag="ctx", bufs=2, name="ctx2")
                            ctx2s.append(ctx2)
                            for i, kb in enumerate(full_kbs):
                                nc.tensor.matmul(
                                    ctx2[:], vnat[:, vbase + kb, :], e_all[:, 1 + i, :, :],
                                    start=(i == 0), stop=False,
                                )
                            if kb_far >= 0:
                                nc.tensor.matmul(
                                    ctx2[:, :, LH], vnat[:, vbase + kb_far, :],
                                    e_all[:, 0, :, LH], start=False, stop=False,
                                )
                            nc.tensor.matmul(
                                ctx2[:, :, RH], vnat[:, vbase + kb_diag, :],
                                e_all[:, 0, :, RH], start=False, stop=True,
                            )

                        # -- denominators + normalize for both head pairs --
                        ctx_sbs = [None] * (HPC // 2)
                        for g in range(HPC // 2):
                            e_all = e_alls[g]
                            den2_t = psB.tile([D, 2, QT], F32, tag="db", bufs=1, name="den2_t")
                            esum = spool.tile([D, (NSL + 1) // 2, 2, QT], BF16, tag="esum", name="esum")
                            cur, n = e_all, nsl
                            while n > 1:
                                h = n // 2
                                nc.vector.tensor_add(
                                    out=esum[:, 0:h], in0=cur[:, 0:h], in1=cur[:, h : 2 * h]
                                )
                                if n % 2:
                                    nc.vector.tensor_add(
                                        out=esum[:, 0:1], in0=esum[:, 0:1],
                                        in1=cur[:, 2 * h : 2 * h + 1],
                                    )
                                cur, n = esum, h
                            nc.tensor.matmul(
                                den2_t[:], onesb_s[:], esum[:, 0, :, :], start=True, stop=True
                            )
                            recf = npool.tile([D, 2, QT], F32, tag="recf", name="recf")
                            nc.vector.reciprocal_approx_fast(recf[:], den2_t[:])
                            ctx_sb = cxpool.tile([D, 2, QT], BF16, tag="ctx_sb", name="ctx_sb")
                            nc.vector.tensor_mul(out=ctx_sb[:], in0=ctx2s[g][:], in1=recf[:])
                            ctx_sbs[g] = ctx_sb

                        pending = (b, t, ctx_sbs)
                emit_outproj(pending)

    nc.finalize()
    _NC_CACHE = nc
    return nc


def _rope_cache_np(position_ids):
    """cos/sin [D, T] transposed rope cache from actual position ids,
    scaled by D^-0.25 (half of 1/sqrt(D) on each of Q and K)."""
    lam = float(D) ** -0.25
    inv_freq = 1.0 / (ROPE_THETA ** (np.arange(0, D, 2, dtype=np.float64) / D))
    cos_parts, sin_parts = [], []
    for b in range(B):
        t = np.asarray(position_ids[b], dtype=np.float64)
        freqs = np.outer(t, inv_freq)                    # [S, D/2]
        emb = np.concatenate([freqs, freqs], axis=-1)    # [S, D]
        cos_parts.append(np.cos(emb).T * lam)
        sin_parts.append(np.sin(emb).T * lam)
    cos = np.ascontiguousarray(np.concatenate(cos_parts, axis=1)).astype(ml_dtypes.bfloat16)
    sin = np.ascontiguousarray(np.concatenate(sin_parts, axis=1)).astype(ml_dtypes.bfloat16)
    return cos, sin


def _mask_tiles_np():
    """[2, 128, 2, 128] additive bias tiles in [k, (head, q)] layout; the same
    mask is duplicated on the head axis so one strided DVE add covers both.

    diag[kl, ql] = 0 if kl <= ql else NEG        (k-block == q-block)
    far[kl, ql]  = 0 if ql <  kl else NEG        (k-block == q-block - 8)
    """
    kl = np.arange(128)[:, None]
    ql = np.arange(128)[None, :]
    diag = np.where(kl <= ql, 0.0, NEG).astype(np.float32)
    far = np.where(ql < kl, 0.0, NEG).astype(np.float32)
    both = np.stack([diag, far])                          # [2, 128, 128]
    return np.repeat(both[:, :, None, :], 2, axis=2).astype(np.float32)


def host_in_maps(hidden_states, Wq, Wk, Wv, Wo, position_ids):
    """Shard + pre-layout the full inputs into 8 per-core input maps."""
    hidden_states = np.asarray(hidden_states, dtype=np.float32)
    Wq = np.asarray(Wq, dtype=np.float32)
    Wk = np.asarray(Wk, dtype=np.float32)
    Wv = np.asarray(Wv, dtype=np.float32)
    Wo = np.asarray(Wo, dtype=np.float32)

    bf = ml_dtypes.bfloat16
    xt = np.ascontiguousarray(hidden_states.reshape(T, H).T).astype(bf)
    cos, sin = _rope_cache_np(np.asarray(position_ids))
    masks = _mask_tiles_np()
    ident = np.eye(D).astype(bf)
    onesb = np.ones((D, D), dtype=bf)

    in_maps = []
    for c in range(N_CORES):
        wqt = np.ascontiguousarray(Wq[c * QD : (c + 1) * QD, :].T).astype(bf)
        wkvt = np.ascontiguousarray(
            np.concatenate(
                [Wk[c * D : (c + 1) * D, :].T, Wv[c * D : (c + 1) * D, :].T], axis=1
            )
        ).astype(bf)
        wot = np.ascontiguousarray(Wo[:, c * QD : (c + 1) * QD].T).astype(bf)
        in_maps.append(
            {
                "xt": xt,
                "wqt": wqt,
                "wkvt": wkvt,
                "wot": wot,
                "cosb": cos,
                "sinb": sin,
                "masks": masks,
                "ident": ident,
                "onesb": onesb,
            }
        )
    return in_maps


def kernel(hidden_states, Wq, Wk, Wv, Wo, position_ids):
    nc = build_nc()
    in_maps = host_in_maps(hidden_states, Wq, Wk, Wv, Wo, position_ids)
    res = run_bass_kernel_spmd(nc, in_maps, core_ids=list(range(N_CORES)))
    total = res.results[0]["out"].astype(np.float32)
    for c in range(1, N_CORES):
        total = total + res.results[c]["out"].astype(np.float32)
    return np.ascontiguousarray(total.reshape(B, S, H), dtype=np.float32)


# revision 22
# speedup vs baseline: 1.0592x; 1.0049x over previous
"""Mistral sliding-window attention (B=2, S=2048, H=4096, 32 q-heads / 8 kv-heads,
head_dim=128, window=1024) on 8 Trainium2 NeuronCores.

Sharding: tensor-parallel over heads. Core c owns q-heads [4c, 4c+4) and kv-head c:
  Wq rows [512c, 512c+512), Wk/Wv rows [128c, 128c+128), Wo cols [512c, 512c+512).
Each core computes a full-shape partial output (its heads' contribution through
Wo, in bf16); the host sums the 8 partials in f32 (standard TP unshard).

Per-core kernel (all-bf16 matmuls):
  Phase A: QKV projections from X.T, RoPE fused on the drain (cross-partition
           DVE shifts for rotate_half; D^-0.25 folded into the cos/sin tables
           for both Q and K). Q, K^T, V (PE-transposed) stay resident in SBUF.
           Wq is DMA'd per h-chunk on the vector queue so the first matmul
           starts immediately; X panels own the sync queue.
  Phase B+C per (batch, 256-token q-tile), transposed layout [k, (head, q)]:
           scores for 8 full k-blocks at full width; the two triangular edge
           blocks (diagonal kb=2t+1, window edge kb=2t-8) run at half width
           via strided APs and share one slot of the per-group P^T mega-tile
           (their live halves are complementary). exp on ACT -> bf16 P^T.
           ctx^T accumulates V^T P^T (edges strided, half width). The softmax
           denominator is a strided log-tree DVE sum over the <=9 P^T slots
           plus one ones-matmul (reduces partitions + broadcasts). Fast
           reciprocal + one merged normalize mul, then the output projection
           for those 256 tokens, bf16 partial out.
"""

import math
import sys

sys.path.insert(0, "/opt/trn_rl_repo")

import ml_dtypes
import numpy as np

import concourse.bass as bass
import concourse.mybir as mybir
import concourse.tile as tile
from concourse import bacc
from concourse.bass_utils import run_bass_kernel_spmd

# Problem constants (hardcoded per contract)
B, S, H = 2, 2048, 4096
N_HEADS, N_KV_HEADS, D = 32, 8, 128
WINDOW = 1024
ROPE_THETA = 10000.0
N_CORES = 8
HPC = N_HEADS // N_CORES          # q heads per core = 4
QD = HPC * D                      # per-core q projection dim = 512
T = B * S                         # flattened tokens = 4096

PW = 512                          # phase-A token panel width
QT = 256                          # phase-B query tile width (2 q-blocks)
NEG = -1.0e30

F32 = mybir.dt.float32
BF16 = mybir.dt.bfloat16
AF = mybir.ActivationFunctionType
ALU = mybir.AluOpType

_NC_CACHE = None


def build_nc():
    """Build (once) the single SPMD Bass program all 8 cores run."""
    global _NC_CACHE
    if _NC_CACHE is not None:
        return _NC_CACHE

    nc = bacc.Bacc(None)

    xt_d = nc.dram_tensor("xt", [H, T], BF16, kind="ExternalInput")
    wqt_d = nc.dram_tensor("wqt", [H, QD], BF16, kind="ExternalInput")
    wkvt_d = nc.dram_tensor("wkvt", [H, 2 * D], BF16, kind="ExternalInput")
    wot_d = nc.dram_tensor("wot", [QD, H], BF16, kind="ExternalInput")
    cos_d = nc.dram_tensor("cosb", [D, T], BF16, kind="ExternalInput")
    sin_d = nc.dram_tensor("sinb", [D, T], BF16, kind="ExternalInput")
    mask_d = nc.dram_tensor("masks", [2, D, 2, D], F32, kind="ExternalInput")
    iden_d = nc.dram_tensor("ident", [D, D], BF16, kind="ExternalInput")
    onesb_d = nc.dram_tensor("onesb", [D, D], BF16, kind="ExternalInput")
    out_d = nc.dram_tensor("out", [T, H], BF16, kind="ExternalOutput")

    HC = H // 128                 # 32 h-chunks
    NPAN = T // PW                # 8 token panels
    NQT = S // QT                 # 8 q-tiles per batch
    QC = QD // 128                # 4 qd chunks == heads per core
    NSL = 9                       # P^T mega-tile slots: 1 edge + 8 full

    with tile.TileContext(nc) as tc, nc.allow_low_precision(reason="mixed dtypes"):
        with (
            tc.tile_pool(name="persist", bufs=1) as ppool,
            tc.tile_pool(name="wopool", bufs=1) as wopool,
            tc.tile_pool(name="bpool", bufs=1) as bpool,
        ):
            # K^T (rope'd), V natural-layout and Q (all bf16) stay in SBUF
            kt_full = ppool.tile([D, T], BF16)
            vnat = ppool.tile([128, T // 128, D], BF16)
            qsb = ppool.tile([D, B, HPC, S], BF16)
            # phase-B constants prefetched up front (overlap with phase A)
            mask_s = bpool.tile([D, 2, 2, D], F32)
            nc.gpsimd.dma_start(mask_s[:], mask_d[:].rearrange("m p t q -> p m t q"))
            onesb_s = bpool.tile([D, D], BF16)
            nc.gpsimd.dma_start(onesb_s[:], onesb_d[:])

            # ---------------- Phase A: QKV projections + RoPE ----------------
            with (
                tc.tile_pool(name="wpool", bufs=1) as wpool,
                tc.tile_pool(name="xpool", bufs=8) as xpool,
                tc.tile_pool(name="cspool", bufs=1) as cspool,
                tc.tile_pool(name="apool", bufs=2) as apool,
                tc.tile_pool(name="psA", bufs=1, space="PSUM") as psA,
            ):
                # weights DMA'd per h-chunk, interleaved with panel-0 X chunks
                # (sync queue) and on gpsimd (wk|wv) so each chunk lands just
                # ahead of its matmuls; first matmul starts almost immediately
                wq_s = wpool.tile([128, HC, QD], BF16)
                wkv_s = wpool.tile([128, HC, 2 * D], BF16)
                cos_s = cspool.tile([D, T], BF16)
                sin_s = cspool.tile([D, T], BF16)
                iden_s = cspool.tile([D, D], BF16)
                nc.gpsimd.dma_start(iden_s[:], iden_d[:])
                wo_s = wopool.tile([128, QC, H], BF16)

                for p in range(NPAN):
                    tok = slice(p * PW, (p + 1) * PW)
                    bp = (p * PW) // S        # batch this panel belongs to
                    ps_q = [
                        psA.tile([128, PW], F32, tag=f"psq{j}", name=f"psq{j}")
                        for j in range(HPC)
                    ]
                    ps_k = psA.tile([128, PW], F32, tag="psk")
                    ps_v = psA.tile([128, PW], F32, tag="psv")
                    for hc in range(HC):
                        if p == 0:
                            nc.scalar.dma_start(
                                wq_s[:, hc, :], wqt_d[hc * 128 : (hc + 1) * 128, :]
                            )
                            nc.gpsimd.dma_start(
                                wkv_s[:, hc, :], wkvt_d[hc * 128 : (hc + 1) * 128, :]
                            )
                        x_c = xpool.tile([128, PW], BF16, tag="x_c")
                        nc.sync.dma_start(
                            x_c[:], xt_d[hc * 128 : (hc + 1) * 128, tok]
                        )
                        st, sp = hc == 0, hc == HC - 1
                        for j in range(HPC):
                            nc.tensor.matmul(
                                ps_q[j][:],
                                wq_s[:, hc, j * 128 : (j + 1) * 128],
                                x_c[:],
                                start=st,
                                stop=sp,
                            )
                        nc.tensor.matmul(ps_k[:], wkv_s[:, hc, 0:D], x_c[:], start=st, stop=sp)
                        nc.tensor.matmul(ps_v[:], wkv_s[:, hc, D:], x_c[:], start=st, stop=sp)
                    if p == 0:
                        # scalar queue after the wq chunks: rope tables (needed
                        # ~40us in) then the Wo prefetch (needed ~350us in)
                        nc.scalar.dma_start(cos_s[:], cos_d[:])
                        nc.scalar.dma_start(sin_s[:], sin_d[:])
                        nc.scalar.dma_start(
                            wo_s[:], wot_d[:].rearrange("(qc p) hh -> p qc hh", p=128)
                        )

                    # RoPE in bf16: psum drain copy first (frees the accumulator),
                    # then 2x/4x-mode bf16 DVE ops. rot(x)[p<64] = -x[p+64]; [p>=64] = x[p-64]
                    def rope_math(ps_t, out_ap):
                        sb = apool.tile([128, PW], BF16, tag="ropesb", bufs=3, name="ropesb")
                        nc.vector.tensor_copy(sb[:], ps_t[:])
                        rot = apool.tile([128, PW], BF16, tag="rot", bufs=3, name="rot")
                        nc.vector.tensor_scalar_mul(rot[0:64, :], sb[64:128, :], -1.0)
                        nc.vector.tensor_copy(rot[64:128, :], sb[0:64, :])
                        prod = apool.tile([128, PW], BF16, tag="prod", bufs=3, name="prod")
                        nc.vector.tensor_mul(out=prod[:], in0=sb[:], in1=cos_s[:, tok])
                        nc.vector.tensor_mul(out=rot[:], in0=rot[:], in1=sin_s[:, tok])
                        nc.vector.tensor_add(out=out_ap, in0=prod[:], in1=rot[:])

                    lo = p * PW - bp * S
                    if p == NPAN - 1:
                        # last panel: K first and Q in reverse so the PSUM
                        # banks phase B's first matmuls land on free earliest
                        rope_math(ps_k, kt_full[:, tok])
                        for j in reversed(range(HPC)):
                            rope_math(ps_q[j], qsb[:, bp, j, lo : lo + PW])
                    else:
                        for j in range(HPC):
                            rope_math(ps_q[j], qsb[:, bp, j, lo : lo + PW])
                        rope_math(ps_k, kt_full[:, tok])

                    # V natural layout via PE transpose (ACT does the psum drain)
                    v_sb = apool.tile([128, PW], BF16, tag="v_sb")
                    nc.scalar.copy(v_sb[:], ps_v[:])
                    for blk in range(PW // 128):
                        tp = psA.tile([D, D], BF16, tag="tp", bufs=2, name="tp")
                        nc.tensor.transpose(
                            tp[:], v_sb[:, blk * 128 : (blk + 1) * 128], iden_s[:]
                        )
                        nc.vector.tensor_copy(vnat[:, p * (PW // 128) + blk, :], tp[:])

            # ------------- Phase B+C: attention + output projection -------------
            # Software-pipelined tiles: tile t's output projection is emitted
            # between tile t+1's scores and ctx so the PE never waits on the
            # exp (ACT) / den-tree+reciprocal (DVE) chains.
            with (
                tc.tile_pool(name="epool", bufs=5) as epool,
                tc.tile_pool(name="spool", bufs=4) as spool,
                tc.tile_pool(name="npool", bufs=4) as npool,
                tc.tile_pool(name="cxpool", bufs=6) as cxpool,
                tc.tile_pool(name="opool", bufs=3) as opool,
                tc.tile_pool(name="psB", bufs=1, space="PSUM") as psB,
            ):
                LH = slice(0, 128)            # q-block 2t cols within a head
                RH = slice(128, QT)           # q-block 2t+1 cols within a head

                def emit_outproj(pend):
                    pb, pt, pctx = pend
                    for tl in range(QT // 128):
                        tok0 = pb * S + pt * QT + tl * 128
                        o_all = opool.tile([128, H], BF16, tag="o_all", bufs=3)
                        for hb in range(H // 512):
                            ps_o = psB.tile([128, 512], F32, tag="ps_o", bufs=2, name="ps_o")
                            for qc in range(QC):
                                nc.tensor.matmul(
                                    ps_o[:],
                                    pctx[qc // 2][:, qc % 2, tl * 128 : (tl + 1) * 128],
                                    wo_s[:, qc, hb * 512 : (hb + 1) * 512],
                                    start=(qc == 0),
                                    stop=(qc == QC - 1),
                                )
                            osl = o_all[:, hb * 512 : (hb + 1) * 512]
                            # drain split ~11 DVE / 5 ACT per tile
                            if (tl * 8 + hb) % 3 == 2:
                                nc.scalar.copy(osl, ps_o[:])
                            else:
                                nc.vector.tensor_copy(osl, ps_o[:])
                        nc.gpsimd.dma_start(out_d[tok0 : tok0 + 128, :], o_all[:])

                pending = None
                for b in range(B):
                    for t in range(NQT):
                        qsl = slice(t * QT, (t + 1) * QT)
                        kb_diag = 2 * t + 1
                        kb_far = 2 * t - 8          # may be < 0 (absent)
                        full_kbs = list(range(max(0, kb_far + 1), kb_diag))
                        nsl = 1 + len(full_kbs)
                        vbase = (b * S) // 128
                        qps = [qsb[:, b, 2 * g : 2 * g + 2, qsl] for g in range(HPC // 2)]
                        e_alls = []

                        # -- scores + masks + exp for both head pairs --
                        for g in range(HPC // 2):
                            qp = qps[g]
                            e_all = epool.tile([D, NSL, 2, QT], BF16, tag="e_all", name="e_all")
                            e_alls.append(e_all)
                            for i, kb in enumerate(full_kbs):
                                s_ps = psB.tile([D, 2, QT], F32, tag="sc", bufs=3, name="s_ps")
                                nc.tensor.matmul(
                                    s_ps[:],
                                    kt_full[:, b * S + kb * 128 : b * S + (kb + 1) * 128],
                                    qp,
                                    start=True,
                                    stop=True,
                                )
                                if kb == 2 * t:
                                    nc.vector.tensor_add(
                                        out=s_ps[:, :, LH], in0=s_ps[:, :, LH],
                                        in1=mask_s[:, 0, :, :],
                                    )
                                elif kb == 2 * t - 7:
                                    nc.vector.tensor_add(
                                        out=s_ps[:, :, RH], in0=s_ps[:, :, RH],
                                        in1=mask_s[:, 1, :, :],
                                    )
                                nc.scalar.activation(e_all[:, 1 + i, :, :], s_ps[:], AF.Exp)

                            # edge k-blocks at half width into shared slot 0:
                            # diag (kb=2t+1) live on RH, far (kb=2t-8) live on LH
                            s_pe = psB.tile([D, 2, QT], F32, tag="sc", bufs=3, name="s_pe")
                            nc.tensor.matmul(
                                s_pe[:, :, RH],
                                kt_full[:, b * S + kb_diag * 128 : b * S + (kb_diag + 1) * 128],
                                qp[:, :, RH],
                                start=True,
                                stop=True,
                            )
                            nc.vector.tensor_add(
                                out=s_pe[:, :, RH], in0=s_pe[:, :, RH],
                                in1=mask_s[:, 0, :, :],
                            )
                            nc.scalar.activation(e_all[:, 0, :, RH], s_pe[:, :, RH], AF.Exp)
                            if kb_far >= 0:
                                s_pf = psB.tile([D, 2, QT], F32, tag="sc", bufs=3, name="s_pf")
                                nc.tensor.matmul(
                                    s_pf[:, :, LH],
                                    kt_full[:, b * S + kb_far * 128 : b * S + (kb_far + 1) * 128],
                                    qp[:, :, LH],
                                    start=True,
                                    stop=True,
                                )
                                nc.vector.tensor_add(
                                    out=s_pf[:, :, LH], in0=s_pf[:, :, LH],
                                    in1=mask_s[:, 1, :, :],
                                )
                                nc.scalar.activation(e_all[:, 0, :, LH], s_pf[:, :, LH], AF.Exp)
                            else:
                                nc.vector.memset(e_all[:, 0, :, LH], 0.0)

                        # -- previous tile's output projection fills the gap --
                        if pending is not None:
                            emit_outproj(pending)

                        # -- ctx accumulation for both head pairs --
                        ctx2s = []
                        for g in range(HPC // 2):
                            e_all = e_alls[g]
                            ctx2 = psB.tile([D, 2, QT], F32, tag="ctx", bufs=2, name="ctx2")
                            ctx2s.append(ctx2)
                            for i, kb in enumerate(full_kbs):
                                nc.tensor.matmul(
                                    ctx2[:], vnat[:, vbase + kb, :], e_all[:, 1 + i, :, :],
                                    start=(i == 0), stop=False,
                                )
                            if kb_far >= 0:
                                nc.tensor.matmul(
                                    ctx2[:, :, LH], vnat[:, vbase + kb_far, :],
                                    e_all[:, 0, :, LH], start=False, stop=False,
                                )
                            nc.tensor.matmul(
                                ctx2[:, :, RH], vnat[:, vbase + kb_diag, :],
                                e_all[:, 0, :, RH], start=False, stop=True,
                            )

                        # -- denominators + normalize for both head pairs --
                        ctx_sbs = [None] * (HPC // 2)
                        for g in range(HPC // 2):
                            e_all = e_alls[g]
                            den2_t = psB.tile([D, 2, QT], F32, tag="db", bufs=1, name="den2_t")
                            esum = spool.tile([D, (NSL + 1) // 2, 2, QT], BF16, tag="esum", name="esum")
                            cur, n = e_all, nsl
                            while n > 1:
                                h = n // 2
                                nc.vector.tensor_add(
                                    out=esum[:, 0:h], in0=cur[:, 0:h], in1=cur[:, h : 2 * h]
                                )
                                if n % 2:
                                    nc.vector.tensor_add(
                                        out=esum[:, 0:1], in0=esum[:, 0:1],
                                        in1=cur[:, 2 * h : 2 * h + 1],
                                    )
                                cur, n = esum, h
                            nc.tensor.matmul(
                                den2_t[:], onesb_s[:], esum[:, 0, :, :], start=True, stop=True
                            )
                            recf = npool.tile([D, 2, QT], F32, tag="recf", name="recf")
                            nc.vector.reciprocal_approx_fast(recf[:], den2_t[:])
                            ctx_sb = cxpool.tile([D, 2, QT], BF16, tag="ctx_sb", name="ctx_sb")
                            nc.vector.tensor_mul(out=ctx_sb[:], in0=ctx2s[g][:], in1=recf[:])
                            ctx_sbs[g] = ctx_sb

                        pending = (b, t, ctx_sbs)
                emit_outproj(pending)

    nc.finalize()
    _NC_CACHE = nc
    return nc


def _rope_cache_np(position_ids):
    """cos/sin [D, T] transposed rope cache from actual position ids,
    scaled by D^-0.25 (half of 1/sqrt(D) on each of Q and K)."""
    lam = float(D) ** -0.25
    inv_freq = 1.0 / (ROPE_THETA ** (np.arange(0, D, 2, dtype=np.float64) / D))
    cos_parts, sin_parts = [], []
    for b in range(B):
        t = np.asarray(position_ids[b], dtype=np.float64)
        freqs = np.outer(t, inv_freq)                    # [S, D/2]
        emb = np.concatenate([freqs, freqs], axis=-1)    # [S, D]
        cos_parts.append(np.cos(emb).T * lam)
        sin_parts.append(np.sin(emb).T * lam)
    cos = np.ascontiguousarray(np.concatenate(cos_parts, axis=1)).astype(ml_dtypes.bfloat16)
    sin = np.ascontiguousarray(np.concatenate(sin_parts, axis=1)).astype(ml_dtypes.bfloat16)
    return cos, sin


def _mask_tiles_np():
    """[2, 128, 2, 128] additive bias tiles in [k, (head, q)] layout; the same
    mask is duplicated on the head axis so one strided DVE add covers both.

    diag[kl, ql] = 0 if kl <= ql else NEG        (k-block == q-block)
    far[kl, ql]  = 0 if ql <  kl else NEG        (k-block == q-block - 8)
    """
    kl = np.arange(128)[:, None]
    ql = np.arange(128)[None, :]
    diag = np.where(kl <= ql, 0.0, NEG).astype(np.float32)
    far = np.where(ql < kl, 0.0, NEG).astype(np.float32)
    both = np.stack([diag, far])                          # [2, 128, 128]
    return np.repeat(both[:, :, None, :], 2, axis=2).astype(np.float32)


def host_in_maps(hidden_states, Wq, Wk, Wv, Wo, position_ids):
    """Shard + pre-layout the full inputs into 8 per-core input maps."""
    hidden_states = np.asarray(hidden_states, dtype=np.float32)
    Wq = np.asarray(Wq, dtype=np.float32)
    Wk = np.asarray(Wk, dtype=np.float32)
    Wv = np.asarray(Wv, dtype=np.float32)
    Wo = np.asarray(Wo, dtype=np.float32)

    bf = ml_dtypes.bfloat16
    xt = np.ascontiguousarray(hidden_states.reshape(T, H).T).astype(bf)
    cos, sin = _rope_cache_np(np.asarray(position_ids))
    masks = _mask_tiles_np()
    ident = np.eye(D).astype(bf)
    onesb = np.ones((D, D), dtype=bf)

    in_maps = []
    for c in range(N_CORES):
        wqt = np.ascontiguousarray(Wq[c * QD : (c + 1) * QD, :].T).astype(bf)
        wkvt = np.ascontiguousarray(
            np.concatenate(
                [Wk[c * D : (c + 1) * D, :].T, Wv[c * D : (c + 1) * D, :].T], axis=1
            )
        ).astype(bf)
        wot = np.ascontiguousarray(Wo[:, c * QD : (c + 1) * QD].T).astype(bf)
        in_maps.append(
            {
                "xt": xt,
                "wqt": wqt,
                "wkvt": wkvt,
                "wot": wot,
                "cosb": cos,
                "sinb": sin,
                "masks": masks,
                "ident": ident,
                "onesb": onesb,
            }
        )
    return in_maps


def kernel(hidden_states, Wq, Wk, Wv, Wo, position_ids):
    nc = build_nc()
    in_maps = host_in_maps(hidden_states, Wq, Wk, Wv, Wo, position_ids)
    res = run_bass_kernel_spmd(nc, in_maps, core_ids=list(range(N_CORES)))
    total = res.results[0]["out"].astype(np.float32)
    for c in range(1, N_CORES):
        total = total + res.results[c]["out"].astype(np.float32)
    return np.ascontiguousarray(total.reshape(B, S, H), dtype=np.float32)
